# revision 1
# baseline (speedup 1.0000x reference)
"""Trainium2 Bass kernel for nn_Classification_4922032521468.

Problem: acts = embeds[activity_index]  (A=512 rows, d=512)
         pairs = concat(acts[ii], acts[jj])  for all i<j (P=130816 pairs)
         out = log_softmax(pairs @ W.T + b)  -> [P, 4]

Key algebra: logits[p, c] = L[i, c] + R'[j, c]  with
  L  = acts @ Wl.T          (Wl = W[:, :512])
  R' = acts @ Wr.T + b      (Wr = W[:, 512:])
so log_softmax needs only lse[i, j] = ln(sum_c e^{L[i,c]} e^{R'[j,c]})
(a K=4 PE matmul of U = e^L rows against V = e^{R'}) and
  out[i, j, c] = L[i, c] + R'[j, c] - lse[i, j].
No 130816x1024 pair tensor is ever built.

Layout: the per-core output plane is computed TRANSPOSED - j on partitions,
(i, c) on the free axis - which makes every term either per-partition
(R', lse) or a partition-broadcast row (L, built once with a K=1 matmul).

Sharding: core k owns i-rows [64k, 64k+64). The same NEFF runs on all 8
cores (SPMD); per-core behavior comes only from per-core DATA:
activity_index is rotated by -64k so each core's own i-rows are gathered
rows 0..63. Each core outputs [512 j, 64 i, 4 c] (j rotated); the host
un-rotates j, transposes, and gathers the triu pairs.
"""

import numpy as np

A = 512  # number of activity tokens
D = 512  # embedding dim
C = 4  # classes
NTOK = 4096  # embeds table rows
RB = 64  # i-rows per core
NCORES = 8

_program = None
_last_results = None  # BassKernelResults from the most recent run (profiling)


def _build_program():
    from contextlib import ExitStack

    import concourse.bacc as bacc
    import concourse.mybir as mybir
    import concourse.tile as tile
    from concourse.bass import IndirectOffsetOnAxis
    from concourse.tile_rust import add_dep_helper

    fp32 = mybir.dt.float32
    i32 = mybir.dt.int32
    AF = mybir.ActivationFunctionType
    SUB = mybir.AluOpType.subtract
    ADD = mybir.AluOpType.add

    nc = bacc.Bacc(
        "TRN2",
        target_bir_lowering=False,
        debug=False,
        enable_asserts=False,
        num_devices=NCORES,
    )

    embeds_h = nc.dram_tensor("embeds", (NTOK, D), fp32, kind="ExternalInput")
    # idxs[p, j] = rotated activity_index[128j + p], int32
    idx_h = nc.dram_tensor("idxs", (128, 4), i32, kind="ExternalInput")
    # wt[d, 8k+0:4] = Wr.T[128k+d, :], wt[d, 8k+4:8] = Wl.T[128k+d, :]
    wt_h = nc.dram_tensor("wt", (128, 32), fp32, kind="ExternalInput")
    # b8 = [b_0..b_3, 0, 0, 0, 0] (bias folds into R via a K=1 matmul)
    b8_h = nc.dram_tensor("b8", (1, 8), fp32, kind="ExternalInput")
    # out[j, 4i + c] (j rotated per core)
    out_h = nc.dram_tensor("out", (A, RB * C), fp32, kind="ExternalOutput")

    ident_h = nc.inline_tensor(np.eye(128, dtype=np.float32), name="ident")

    embeds_ap = embeds_h.ap()
    out_ap = out_h.ap()

    with tile.TileContext(nc) as tc, ExitStack() as ctx:
        sb = ctx.enter_context(tc.tile_pool(name="sb", bufs=1))
        sbr = ctx.enter_context(tc.tile_pool(name="sbr", bufs=6))
        psT = ctx.enter_context(tc.tile_pool(name="psT", bufs=3, space="PSUM"))
        psR = ctx.enter_context(tc.tile_pool(name="psR", bufs=2, space="PSUM"))
        psB = ctx.enter_context(tc.tile_pool(name="psB", bufs=1, space="PSUM"))
        psS = ctx.enter_context(tc.tile_pool(name="psS", bufs=1, space="PSUM"))

        # ---- gather path first: idx load, then the 4 indirect gathers ----
        idxs = sb.tile([128, 4], i32, tag="idxs")
        nc.sync.dma_start(out=idxs[:], in_=idx_h.ap()[:])

        acts = []
        for j in range(4):
            aj = sb.tile([128, D], fp32, tag=f"acts{j}", name=f"acts{j}")
            nc.gpsimd.indirect_dma_start(
                out=aj[:],
                out_offset=None,
                in_=embeds_ap[:],
                in_offset=IndirectOffsetOnAxis(ap=idxs[:, j : j + 1], axis=0),
            )
            acts.append(aj)

        # ---- small constants (dispatch behind idx on the sync queue) ----
        ident = sb.tile([128, 128], fp32, tag="ident")
        nc.sync.dma_start(out=ident[:], in_=ident_h.ap()[:])
        wt = sb.tile([128, 32], fp32, tag="wt")
        nc.sync.dma_start(out=wt[:], in_=wt_h.ap()[:])
        b4 = sb.tile([C, 1], fp32, tag="b4")
        nc.sync.dma_start(out=b4[:], in_=b8_h.ap()[0:1, 0:C])
        ones = sb.tile([1, 128], fp32, tag="ones")
        nc.vector.memset(ones[:], 1.0)

        # persistent tiles
        rj = sb.tile([128, 16], fp32, tag="rj")  # R' row-major, chunk j cols 4j:4j+4
        rt = sb.tile([C, A], fp32, tag="rt")  # R' transposed
        vt = sb.tile([C, A], fp32, tag="vt")  # e^{R'} transposed (classes on K)
        ut4 = sb.tile([C, RB], fp32, tag="ut4")  # e^{L} transposed
        lt4 = sb.tile([C, RB], fp32, tag="lt4")  # L transposed
        lbf = sb.tile([1, RB * C], fp32, tag="lbf")  # L flattened (4i + c)

        # ---- phase A per j-chunk: transpose, R' matmuls, e^{R'} ----
        # (all Exp ops are emitted before any Ln so the ACT table loads once
        # per function instead of thrashing Exp<->Ln. Matmuls keep the tiny
        # wt as the STATIONARY operand - a [128, 128] stationary would pay a
        # ~1.3us weight load per call.)
        for j in range(4):
            aT = []
            for k in range(4):
                pt = psT.tile([128, 128], fp32, tag="pt", name="pt")
                nc.tensor.transpose(
                    out=pt[:],
                    in_=acts[j][:, 128 * k : 128 * k + 128],
                    identity=ident[:],
                )
                at = sbr.tile([128, 128], fp32, tag="aT", name="aT")
                nc.vector.tensor_copy(out=at[:], in_=pt[:])
                aT.append(at)

            # R'^T chunk [4, 128] = sum_k Wr.T_k.T @ aT_k  (+ b outer ones)
            pr = psR.tile([C, 128], fp32, tag="pr", name="pr")
            for k in range(4):
                nc.tensor.matmul(
                    out=pr[:],
                    lhsT=wt[:, 8 * k : 8 * k + 4],
                    rhs=aT[k][:],
                    start=(k == 0),
                    stop=(k == 3),
                )
            # b rides for free: ACT bias on the exp, DVE scalar-add on rt
            # (classes sit on partitions here, so b is a [4, 1] per-partition
            # operand) - no K=1 PE matmul needed.
            nc.vector.tensor_scalar_add(
                rt[:, 128 * j : 128 * (j + 1)], pr[:], b4[:]
            )
            last_exp = nc.scalar.activation(
                out=vt[:, 128 * j : 128 * (j + 1)],
                in_=pr[:],
                func=AF.Exp,
                bias=b4[:],
            )
            # row-major chunk for the final per-partition add
            prj = psT.tile([128, C], fp32, tag="pt", name="prj")
            nc.tensor.transpose(
                out=prj[:],
                in_=rt[:, 128 * j : 128 * (j + 1)],
                identity=ident[0:C, 0:C],
            )
            nc.vector.tensor_copy(out=rj[:, 4 * j : 4 * j + 4], in_=prj[:])

            if j == 0:
                # L^T [4, 64] (no bias; b lives on the R side)
                pl = psR.tile([C, RB], fp32, tag="pl", name="pl", bufs=1)
                for k in range(4):
                    nc.tensor.matmul(
                        out=pl[:],
                        lhsT=wt[:, 8 * k + 4 : 8 * k + 8],
                        rhs=aT[k][:, 0:RB],
                        start=(k == 0),
                        stop=(k == 3),
                    )
                nc.scalar.activation(out=ut4[:], in_=pl[:], func=AF.Exp)
                nc.vector.tensor_copy(out=lt4[:], in_=pl[:])
                # lbf[0, 4i+c] = L[i, c] via per-class reordering DMAs
                lbf3 = lbf[:].rearrange("o (i c) -> o i c", c=C)
                for c in range(C):
                    nc.sync.dma_start(
                        out=lbf3[:, :, c : c + 1], in_=lt4[c : c + 1, :]
                    )

        # L broadcast across all 128 partitions via K=1 matmul (kept in PSUM)
        lbb = psB.tile([128, RB * C], fp32, tag="lbb")
        nc.tensor.matmul(out=lbb[:], lhsT=ones[:], rhs=lbf[:], start=True, stop=True)
        lbb3 = lbb[:].rearrange("p (i c) -> p i c", c=C)

        # ---- phase B per j-chunk: lse, combine, store ----
        for j in range(4):
            se = psS.tile([128, RB], fp32, tag="se", name="se")
            nc.tensor.matmul(
                out=se[:],
                lhsT=vt[:, 128 * j : 128 * (j + 1)],
                rhs=ut4[:],
                start=True,
                stop=True,
            )
            lnse = sbr.tile([128, RB], fp32, tag="lnse", name="lnse")
            ln_inst = nc.scalar.activation(out=lnse[:], in_=se[:], func=AF.Ln)
            # keep every Ln after the last Exp so the ACT function table
            # loads exactly twice instead of thrashing Exp<->Ln per chunk
            add_dep_helper(
                ln_inst.ins, last_exp.ins, sync=False, reason="act-table order"
            )

            tmp = sbr.tile([128, RB * C], fp32, tag="tmp", name="tmp")
            nc.vector.tensor_tensor(
                out=tmp[:].rearrange("p (i c) -> p i c", c=C),
                in0=lbb3,
                in1=lnse[:].unsqueeze(2).to_broadcast([128, RB, C]),
                op=SUB,
            )
            oj = sbr.tile([128, RB * C], fp32, tag="oj", name="oj")
            nc.vector.tensor_tensor(
                out=oj[:].rearrange("p (i c) -> p i c", c=C),
                in0=tmp[:].rearrange("p (i c) -> p i c", c=C),
                in1=rj[:, 4 * j : 4 * j + 4].unsqueeze(1).to_broadcast([128, RB, C]),
                op=ADD,
            )
            nc.sync.dma_start(
                out=out_ap[128 * j : 128 * (j + 1), :], in_=oj[:]
            )

    nc.compile()
    return nc


def _get_program():
    global _program
    if _program is None:
        _program = _build_program()
    return _program


def _prep_core_inputs(embeds, idx64, wt_np, b8_np, k):
    rot = np.roll(idx64, -RB * k)
    idxs = np.ascontiguousarray(rot.reshape(4, 128).T.astype(np.int32))
    return {"embeds": embeds, "idxs": idxs, "wt": wt_np, "b8": b8_np}


def kernel(embeds, activity_index, W, b):
    from concourse.bass_utils import run_bass_kernel_spmd

    embeds = np.ascontiguousarray(np.asarray(embeds), dtype=np.float32)
    W = np.asarray(W, dtype=np.float32)
    b_in = np.asarray(b, dtype=np.float32).reshape(C)
    idx64 = np.asarray(activity_index).astype(np.int64)

    # wt[d, 8k+0:4] = Wr.T chunk k, wt[d, 8k+4:8] = Wl.T chunk k
    wt_np = np.empty((128, 32), dtype=np.float32)
    for k in range(4):
        wt_np[:, 8 * k : 8 * k + 4] = W[:, D + 128 * k : D + 128 * (k + 1)].T
        wt_np[:, 8 * k + 4 : 8 * k + 8] = W[:, 128 * k : 128 * (k + 1)].T
    wt_np = np.ascontiguousarray(wt_np)
    b8_np = np.zeros((1, 8), dtype=np.float32)
    b8_np[0, 0:C] = b_in

    nc = _get_program()
    in_maps = [
        _prep_core_inputs(embeds, idx64, wt_np, b8_np, k) for k in range(NCORES)
    ]

    results = run_bass_kernel_spmd(nc, in_maps, core_ids=list(range(NCORES)))
    global _last_results
    _last_results = results

    out_sq = np.empty((A, A, C), dtype=np.float32)
    for k in range(NCORES):
        # blk[j, i, c] with j rotated by -64k -> un-rotate and transpose
        blk = results.results[k]["out"].reshape(A, RB, C).transpose(1, 0, 2)
        out_sq[RB * k : RB * (k + 1)] = np.roll(blk, RB * k, axis=1)

    ii, jj = np.triu_indices(A, k=1)
    return np.ascontiguousarray(out_sq[ii, jj])



# revision 14
# speedup vs baseline: 1.0129x; 1.0129x over previous
"""Trainium2 Bass kernel for nn_Classification_4922032521468.

Problem: acts = embeds[activity_index]  (A=512 rows, d=512)
         pairs = concat(acts[ii], acts[jj])  for all i<j (P=130816 pairs)
         out = log_softmax(pairs @ W.T + b)  -> [P, 4]

Algebra: logits[p, c] = L[i, c] + R'[j, c]  with
  L  = acts @ Wl.T          (Wl = W[:, :512])
  R' = acts @ Wr.T + b      (Wr = W[:, 512:])
log_softmax needs lse[j, i] = ln(sum_c e^{R'[j,c]} e^{L[i,c]}) (K=4 PE
matmul of V = e^{R'} against U = e^{L}) and
  out[j, i, c] = L[i, c] + R'[j, c] - lse[j, i].
No 130816x1024 pair tensor is ever built.

Fast path vs the 42us fp32 version:
 - fp16 data path end-to-end (PE 4x faster than fp32; DMA bytes halved;
   output cast back to fp32 on host). Headroom: harness gate is 2e-2,
   this lands ~1e-3.
 - ONE dma_gather(transpose=True): gathers all 512 rows AND transposes
   them into [128 d, 4 dchunk, 512 j] in a single SWDGE instruction -
   replaces 4 indirect gathers + 16 PE transposes + 16 PSUM copies.
 - R'^T and L^T fall out of 4 wide matmuls (N=512) into one PSUM bank.
 - One [8, 512] exp produces e^{R'+b} (rows 0:4) and e^{L} (rows 4:8).
 - L broadcast built without DMA hops: rhsL[c, (i,c')] = L[i,c]*I4[c,c']
   via one DVE op, then ones4^T @ rhsL = L broadcast to 128 partitions.
 - A manual LoadActFuncSet of a table set containing BOTH Exp and Ln is
   issued before any activation, so no ACT table switch sits on the
   critical path between exp and ln.
 - Fused fp16 combines ([128, 1024] in 2 DVE ops) and one output DMA.

Sharding: core k owns i-rows [64k, 64k+64). Same NEFF on all 8 cores;
per-core behavior via data only: activity_index rotated by -64k so core
k's i-rows are gathered rows 0..63. Each core outputs [512 j, 64 i, 4 c]
(j rotated); host un-rotates j, transposes, gathers the triu pairs.
"""

import numpy as np

A = 512  # number of activity tokens
D = 512  # embedding dim
C = 4  # classes
NTOK = 4096  # embeds table rows
RB = 64  # i-rows per core
NCORES = 8

_program = None
_last_results = None


def _build_program(debug_dump=False):
    from contextlib import ExitStack

    import concourse.bacc as bacc
    import concourse.mybir as mybir
    import concourse.tile as tile
    from concourse.tile_rust import add_dep_helper

    fp32 = mybir.dt.float32
    fp16 = mybir.dt.float16
    i16 = mybir.dt.int16
    AF = mybir.ActivationFunctionType
    SUB = mybir.AluOpType.subtract
    ADD = mybir.AluOpType.add
    MUL = mybir.AluOpType.mult

    nc = bacc.Bacc(
        "TRN2",
        target_bir_lowering=False,
        debug=False,
        enable_asserts=False,
        num_devices=NCORES,
    )

    emb16_h = nc.dram_tensor("emb16", (NTOK, D), fp16, kind="ExternalInput")
    # idx16[p, s] = rotated activity_index[16s + p] for p < 16; rows 16:128 zero
    idx_h = nc.dram_tensor("idx16", (128, 32), i16, kind="ExternalInput")
    # wt[d, 8k+0:4] = Wr.T[128k+d, :], wt[d, 8k+4:8] = Wl.T[128k+d, :]
    wt_h = nc.dram_tensor("wt16", (128, 32), fp16, kind="ExternalInput")
    # b as a per-partition column
    b4_h = nc.dram_tensor("b4v", (C, 1), fp32, kind="ExternalInput")
    # out[j, 4i + c] fp16 (j rotated per core); host casts to fp32
    out_h = nc.dram_tensor("out", (A, RB * C), fp16, kind="ExternalOutput")

    i4_h = nc.inline_tensor(np.eye(C, dtype=np.float16), name="i4")

    with tile.TileContext(nc) as tc, ExitStack() as ctx:
        sb = ctx.enter_context(tc.tile_pool(name="sb", bufs=1))
        psA = ctx.enter_context(tc.tile_pool(name="psA", bufs=1, space="PSUM"))
        psB = ctx.enter_context(tc.tile_pool(name="psB", bufs=1, space="PSUM"))
        psC = ctx.enter_context(tc.tile_pool(name="psC", bufs=1, space="PSUM"))
        psD = ctx.enter_context(tc.tile_pool(name="psD", bufs=1, space="PSUM"))
        psE = ctx.enter_context(tc.tile_pool(name="psE", bufs=1, space="PSUM"))

        # ---- index load first (critical path), then the fused
        # gather+transpose: aT3[p, k, j] = emb16[idx_j, 128k + p] ----
        idxs = sb.tile([128, 32], i16, tag="idxs")
        nc.sync.dma_start(out=idxs[:], in_=idx_h.ap()[:])

        aT3 = sb.tile([128, 4 * D], fp16, tag="aT3")
        aT3v = aT3[:].rearrange("p (k j) -> p k j", k=4)
        nc.gpsimd.dma_gather(
            out_ap=aT3v,
            in_ap=emb16_h.ap()[:],
            idxs_ap=idxs[:],
            num_idxs=A,
            num_idxs_reg=A,
            elem_size=D,
            transpose=True,
        )

        # ---- small constants (off critical path) ----
        wt = sb.tile([128, 32], fp16, tag="wt")
        nc.scalar.dma_start(out=wt[:], in_=wt_h.ap()[:])
        b4v = sb.tile([C, 1], fp32, tag="b4v")
        nc.scalar.dma_start(out=b4v[:], in_=b4_h.ap()[:])
        i4 = sb.tile([C, C], fp16, tag="i4")
        nc.sync.dma_start(out=i4[:], in_=i4_h.ap()[:])
        ones4 = sb.tile([C, 128], fp16, tag="ones4")
        nc.vector.memset(ones4[:], 1.0)

        # ---- ACT table: load a set that has BOTH Exp and Ln, before any
        # activation, so no table switch lands mid-kernel ----
        load_inst = None
        try:
            from concourse.hw_specs import get_activation_tables

            tables = get_activation_tables(nc.m.arch)
            set_id = None
            for i, (name, funcs) in enumerate(tables.items()):
                if AF.Exp in funcs and AF.Ln in funcs:
                    set_id = i
                    break
            if set_id is not None:
                li = mybir.InstLoadActFuncSet(
                    name=nc.get_next_instruction_name(),
                    ins=[],
                    outs=[],
                    act_func_set_id=set_id,
                )
                load_inst = nc.scalar.add_instruction(li)
        except Exception:
            load_inst = None

        # ---- phase A: R'^T [4, 512] and L^T [4, 64], both base partition 0 ----
        pr = psA.tile([C, D], fp32, tag="pr")
        for k in range(4):
            nc.tensor.matmul(
                out=pr[:],
                lhsT=wt[:, 8 * k : 8 * k + 4],
                rhs=aT3v[:, k, :],
                start=(k == 0),
                stop=(k == 3),
            )
        pl = psE.tile([C, RB], fp32, tag="pl")
        for k in range(4):
            nc.tensor.matmul(
                out=pl[:],
                lhsT=wt[:, 8 * k + 4 : 8 * k + 8],
                rhs=aT3v[:, k, 0:RB],
                start=(k == 0),
                stop=(k == 3),
            )

        # uv[:, 0:512] = e^{R'+b} (V), uv[:, 512:576] = e^{L} (U)  (fp16)
        uv = sb.tile([C, D + RB], fp16, tag="uv")
        exp_inst = nc.scalar.activation(
            out=uv[:, 0:D], in_=pr[:], func=AF.Exp, bias=b4v[:]
        )
        if load_inst is not None:
            add_dep_helper(exp_inst.ins, load_inst.ins, sync=False, reason="act-table")
        nc.scalar.activation(out=uv[:, D : D + RB], in_=pl[:], func=AF.Exp)

        # rall[c, j] = R' + b (fp16, row-major source for rjt transposes)
        rall = sb.tile([C, D], fp16, tag="rall")
        nc.vector.tensor_scalar_add(rall[:], pr[:], b4v[:])

        # ---- L broadcast: lbb[p, (i, c)] = L[i, c] for all p ----
        # rhsL[c, (i, c')] = L[i, c] * I4[c, c']; lbb = ones4^T @ rhsL
        rhsL = sb.tile([C, RB * C], fp16, tag="rhsL")
        nc.vector.tensor_tensor(
            out=rhsL[:].rearrange("c (i cc) -> c i cc", cc=C),
            in0=pl[:].unsqueeze(2).to_broadcast([C, RB, C]),
            in1=i4[:].unsqueeze(1).to_broadcast([C, RB, C]),
            op=MUL,
        )
        lbb = psB.tile([128, RB * C], fp32, tag="lbb")
        nc.tensor.matmul(out=lbb[:], lhsT=ones4[:], rhs=rhsL[:], start=True, stop=True)
        lbb3 = lbb[:].rearrange("p (i c) -> p i c", c=C)

        # ---- R' row-major: rjt[:, 4g:4g+4] = rall[:, 128g:+128]^T ----
        rjt = psC.tile([128, 4 * C], fp16, tag="rjt")
        for g in range(4):
            nc.tensor.transpose(
                out=rjt[:, C * g : C * (g + 1)],
                in_=rall[:, 128 * g : 128 * (g + 1)],
                identity=i4[:],
            )

        # ---- lse: se[jl, 64g + i] = sum_c e^{R'[j]+b} e^{L[i]} ----
        se = psD.tile([128, 4 * RB], fp32, tag="se")
        for g in range(4):
            nc.tensor.matmul(
                out=se[:, RB * g : RB * (g + 1)],
                lhsT=uv[:, 128 * g : 128 * (g + 1)],
                rhs=uv[:, D : D + RB],
                start=True,
                stop=True,
            )
        lnse = sb.tile([128, 4 * RB], fp16, tag="lnse")
        nc.scalar.activation(out=lnse[:], in_=se[:], func=AF.Ln)

        # ---- combine: o[p, g, i, c] = L[i,c] - lnse[p, 64g+i] + R'[j,c]+b ----
        tall = sb.tile([128, 4 * RB * C], fp16, tag="tall")
        tall4 = tall[:].rearrange("p (g i c) -> p g i c", g=4, c=C)
        nc.vector.tensor_tensor(
            out=tall4,
            in0=lbb3.unsqueeze(1).to_broadcast([128, 4, RB, C]),
            in1=lnse[:]
            .rearrange("p (g i) -> p g i", g=4)
            .unsqueeze(3)
            .to_broadcast([128, 4, RB, C]),
            op=SUB,
        )
        oall = sb.tile([128, 4 * RB * C], fp16, tag="oall")
        nc.vector.tensor_tensor(
            out=oall[:].rearrange("p (g i c) -> p g i c", g=4, c=C),
            in0=tall4,
            in1=rjt[:]
            .rearrange("p (g c) -> p g c", g=4)
            .unsqueeze(2)
            .to_broadcast([128, 4, RB, C]),
            op=ADD,
        )

        # ---- store: out row 128g + p = oall[p, g, :] ----
        nc.sync.dma_start(
            out=out_h.ap().rearrange("(g p) f -> p g f", g=4),
            in_=oall[:].rearrange("p (g f) -> p g f", g=4),
        )

        if debug_dump:
            d_aT3 = nc.dram_tensor("d_aT3", (128, 4 * D), fp16, kind="ExternalOutput")
            nc.sync.dma_start(out=d_aT3.ap()[:], in_=aT3[:])
            d_uv = nc.dram_tensor("d_uv", (C, D + RB), fp16, kind="ExternalOutput")
            nc.sync.dma_start(out=d_uv.ap()[:], in_=uv[:])
            d_rall = nc.dram_tensor("d_rall", (C, D), fp16, kind="ExternalOutput")
            nc.sync.dma_start(out=d_rall.ap()[:], in_=rall[:])
            d_lnse = nc.dram_tensor("d_lnse", (128, 4 * RB), fp16, kind="ExternalOutput")
            nc.sync.dma_start(out=d_lnse.ap()[:], in_=lnse[:])
            d_rhsL = nc.dram_tensor("d_rhsL", (C, RB * C), fp16, kind="ExternalOutput")
            nc.sync.dma_start(out=d_rhsL.ap()[:], in_=rhsL[:])
            d_tall = nc.dram_tensor("d_tall", (128, 4 * RB * C), fp16, kind="ExternalOutput")
            nc.sync.dma_start(out=d_tall.ap()[:], in_=tall[:])

    nc.compile()
    return nc


def _get_program():
    global _program
    if _program is None:
        _program = _build_program()
    return _program


def _prep_core_inputs(emb16, idx64, wt_np, b4_np, k):
    rot = np.roll(idx64, -RB * k)
    # idx i lives at [i % 16, i // 16]; the 16-partition block is replicated
    # to all 8 Q7-core partition groups (HW reads per-core groups).
    blk = rot.reshape(32, 16).T.astype(np.int16)
    idx16 = np.ascontiguousarray(np.tile(blk, (8, 1)))
    return {"emb16": emb16, "idx16": idx16, "wt16": wt_np, "b4v": b4_np}


def kernel(embeds, activity_index, W, b):
    from concourse.bass_utils import run_bass_kernel_spmd

    emb16 = np.ascontiguousarray(np.asarray(embeds, dtype=np.float32).astype(np.float16))
    W = np.asarray(W, dtype=np.float32)
    b_in = np.asarray(b, dtype=np.float32).reshape(C)
    idx64 = np.asarray(activity_index).astype(np.int64)

    wt_np = np.empty((128, 32), dtype=np.float16)
    for k in range(4):
        wt_np[:, 8 * k : 8 * k + 4] = W[:, D + 128 * k : D + 128 * (k + 1)].T
        wt_np[:, 8 * k + 4 : 8 * k + 8] = W[:, 128 * k : 128 * (k + 1)].T
    wt_np = np.ascontiguousarray(wt_np)
    b4_np = np.ascontiguousarray(b_in.reshape(C, 1))

    nc = _get_program()
    in_maps = [
        _prep_core_inputs(emb16, idx64, wt_np, b4_np, k) for k in range(NCORES)
    ]

    results = run_bass_kernel_spmd(nc, in_maps, core_ids=list(range(NCORES)))
    global _last_results
    _last_results = results

    out_sq = np.empty((A, A, C), dtype=np.float32)
    for k in range(NCORES):
        blk = (
            results.results[k]["out"]
            .astype(np.float32)
            .reshape(A, RB, C)
            .transpose(1, 0, 2)
        )
        out_sq[RB * k : RB * (k + 1)] = np.roll(blk, RB * k, axis=1)

    ii, jj = np.triu_indices(A, k=1)
    return np.ascontiguousarray(out_sq[ii, jj])
